# revision 40
# baseline (speedup 1.0000x reference)
"""Trainium2 Bass kernel for nn_MemoryModule (attention read over a memory bank).

reference:  logits = x @ mem^T ; attn = softmax(logits, axis=1) ; out = attn @ mem
shapes:     x [32768, 128], mem [4096, 128] -> out [32768, 128]

Sharding: data-parallel over batch across 8 cores (4096 rows each), memory
replicated.  No collectives needed (forward only).

Per-core algorithm (B=4096 local rows, M=4096, D=128), for each group of
NB=512 batch columns:
  - Keep XT [d=128, b] and memT [d=128, m] in SBUF as f32r (PE transposes
    f32 at 2 cyc/row; the DVE copy out of PSUM performs the official
    round-to-f32r the HW verifier demands); mem_nat [m%128, chunk, d] in
    bf16 for mm2 (matmul dtype rule: f32/f32r may not mix with bf16).
  - mm1 (f32r, 1 cyc/row): logit TRIADS (3 m-chunks x 512 batch) stream
    through a 2-buf PSUM pool of 3-bank tiles; the ACT engine
    exponentiates a whole triad in one 1536-elem instruction (amortizing
    its ~185ns per-instruction access latency -- ACT is the bottleneck at
    ~127us busy).  Two bufs give exactly one instruction of slack: mm1 of
    triad t+1 runs entirely under ACT of triad t, zero ACT bubbles.
    Chunks 30,31 of each group form a trailing DUO (1024-elem ACT op).
  - exp uses a fixed bias of -45 (constant cancels in softmax); PT is
    written in bf16 (range needs bf16: biased logits span e^-90..e^+24).
  - mm2 (bf16, 1 cyc/row): outT[d, b] += mem_chunk.T @ PT_chunk, PSUM
    accumulate over all 32 chunks.  mm2 is emitted lagging mm1/ACT by 3
    triads (queue carries across group boundaries, no pop in the duo
    iteration) so the PE's static order keeps mm1 ahead of ACT-dependent
    work everywhere, including at group boundaries.
  - sumexp: PT triads are summed on DVE in bf16 (2x perf mode; two
    alternating accumulators, or a single chain in the last group to
    shorten the tail), folded to [128, 512] f32, then one gpsimd
    partition_all_reduce per group yields the broadcast sumexp.  This
    replaces the baseline's ones-matmul, cutting PE work by a third.
  - finalize: DVE reciprocal; outT *= rsum; PE transpose back to [b, d];
    DMA out.  The sum chain for group g is emitted at t==2 of group g+1
    (right after the queued mm2 flush), the out chain at t==8, so the PE
    never stalls on the ACT->DVE->gpsimd chain; the last group's finalize
    runs in two column halves to pipeline the tail.
  - PSUM: triad pool 2x3 banks + outT 1 bank + transpose scratch 1 bank.
  - Startup: DMA slices ordered x0, m0..m7, x1..x7 (the shared DMA path
    serializes at ~790ns/slice, sem prop +900ns); the first two prep
    transposes borrow the idle triad-pool banks, m-slices 2..7 interleave
    into group 0's triad stream, and x-slices 2..7 are built one per
    group at t==5 where the PE has slack.  First matmul fires ~2.5us in.
Cost-model time: ~139.3us vs 197.3us for the ones-matmul baseline.
"""

import numpy as np

import concourse.bass as bass
import concourse.bass_isa as bass_isa
import concourse.mybir as mybir
import concourse.tile as tile
from concourse import bacc
from concourse.masks import make_identity

B, M, D = 32768, 4096, 128
NCORES = 8
BLOC = B // NCORES  # 4096 rows per core
P = 128
NB = 512            # batch columns per group (f32 moving-operand max)
NG = BLOC // NB     # 8 groups
MCHUNKS = M // P    # 32
TRI = 3             # m-chunks per LT slot / ACT op
NTRI = 10           # full triads per group (chunks 0..29)

F32 = mybir.dt.float32
F32R = mybir.dt.float32r
BF16 = mybir.dt.bfloat16


def _r(ap):
    """View an f32 AP as f32r for full-rate (1 cycle/row) matmul."""
    return ap.bitcast(F32R)


def build_nc():
    nc = bacc.Bacc(
        "TRN2", target_bir_lowering=False, debug=False, enable_asserts=False
    )
    x = nc.dram_tensor("x", [BLOC, D], F32, kind="ExternalInput").ap()
    mem = nc.dram_tensor("mem", [M, D], F32, kind="ExternalInput").ap()
    out = nc.dram_tensor("out", [BLOC, D], F32, kind="ExternalOutput").ap()

    with tile.TileContext(nc) as tc:
        with (
            tc.tile_pool(name="const", bufs=1) as constp,
            tc.tile_pool(name="pt", bufs=6) as ptp,
            tc.tile_pool(name="tr", bufs=4) as trp,
            tc.tile_pool(name="fold", bufs=2) as foldp,
            tc.tile_pool(name="sb", bufs=3) as sbp,
            tc.tile_pool(name="psq", bufs=2, space="PSUM") as psq,
            tc.tile_pool(name="pso", bufs=1, space="PSUM") as pso,
            tc.tile_pool(name="psb", bufs=1, space="PSUM") as psb,
        ):
            ident = constp.tile([P, P], F32)
            make_identity(nc, ident)
            expbias = constp.tile([P, 1], F32)
            nc.vector.memset(expbias, -45.0)
            # preload the ACT exp table (1.3us) while DMA streams in
            dummy = constp.tile([P, 1], BF16)
            nc.scalar.activation(
                dummy, expbias, mybir.ActivationFunctionType.Exp, bias=expbias
            )

            # Natural-layout staging tiles: partition = row%128, free = (chunk, d)
            stage_m = constp.tile([P, MCHUNKS, D], F32)
            mem_t = mem.rearrange("(c p) d -> p c d", p=P)
            stage_x = constp.tile([P, BLOC // P, D], F32)
            x_t = x.rearrange("(t p) d -> p t d", p=P)

            def dma_m(s):
                sl = slice(4 * s, 4 * s + 4)
                nc.sync.dma_start(out=stage_m[:, sl, :], in_=mem_t[:, sl, :])

            def dma_x(s):
                sl = slice(4 * s, 4 * s + 4)
                nc.sync.dma_start(out=stage_x[:, sl, :], in_=x_t[:, sl, :])

            dma_m(0)
            dma_x(0)
            for s in range(1, MCHUNKS // 4):
                dma_m(s)
            for s in range(1, BLOC // P // 4):
                dma_x(s)

            # bf16 copy of mem for mm2: the HW verifier requires matching
            # matmul input dtypes when f32/f32r is involved, so the
            # stationary must be bf16 to pair with the bf16 PT moving
            mem_nat = constp.tile([P, MCHUNKS, D], BF16)
            memT = constp.tile([P, M], F32R)
            XT = constp.tile([P, BLOC], F32R)

            def build_slice(src, dst, s, pool=None):
                """PE-transpose slice s (4 tiles of [P, P]) into dst columns.

                The 1-bank psb pool serializes consecutive transposes
                against their DVE copies; prep pace tolerates that."""
                tp = (pool or psb).tile([P, 4 * P], F32, tag="tp" if pool is None else "lt")
                for j in range(4):
                    nc.tensor.transpose(
                        tp[:, j * P : (j + 1) * P],
                        src[:, 4 * s + j, :],
                        ident,
                    )
                nc.vector.tensor_copy(
                    out=dst[:, s * 4 * P : (s + 1) * 4 * P], in_=tp
                )
                if src is stage_m:
                    nc.vector.tensor_copy(
                        out=mem_nat[:, 4 * s : 4 * s + 4, :],
                        in_=stage_m[:, 4 * s : 4 * s + 4, :],
                    )

            # prep only what group 0 needs soon; the rest interleaves into
            # group 0's triad stream below.  The first two slices borrow
            # the two idle LT slots so they don't serialize through psb.
            build_slice(stage_m, memT, 0, pool=psq)
            build_slice(stage_x, XT, 0, pool=psq)
            build_slice(stage_m, memT, 1)

            # deferred output de-transpose state: (g, outs_sb) per group
            pending = [None]

            def emit_sum_chain(g, outT, s_f32):
                """sumexp -> rsum -> scaled outT, emitted at group end so
                the chain completes before group g+1's mm2 needs the
                single-buffered accumulator bank."""
                # partition all-reduce (includes broadcast): [128, NB] f32
                sr = sbp.tile([P, NB], F32, tag="sr")
                nc.gpsimd.partition_all_reduce(
                    sr, s_f32, channels=P, reduce_op=bass_isa.ReduceOp.add
                )
                rbc = sbp.tile([P, NB], F32, tag="rbc")
                nc.vector.reciprocal(rbc, sr)
                outs_sb = sbp.tile([P, NB], F32, tag="outs")
                nc.vector.tensor_mul(outs_sb, outT, rbc)
                pending[0] = (g, outs_sb)

            def emit_out_chain():
                if pending[0] is None:
                    return
                g, outs_sb = pending[0]
                pending[0] = None
                onat = psb.tile([P, NB], F32, tag="tp")
                for j in range(NB // P):
                    nc.tensor.transpose(
                        onat[:, j * P : (j + 1) * P],
                        outs_sb[:, j * P : (j + 1) * P],
                        ident,
                    )
                out_sb = sbp.tile([P, NB], F32, tag="osb")
                nc.vector.tensor_copy(out=out_sb, in_=onat)
                nc.sync.dma_start(
                    out=out[g * NB : (g + 1) * NB, :].rearrange(
                        "(j p) d -> p j d", p=P
                    ),
                    in_=out_sb.rearrange("p (j d) -> p j d", d=D),
                )

            # group-0 injection schedule: m-slices (needed by group 0's own
            # later triads) plus x-slice 1; x-slices 3..8 spread one per
            # group at t==5 where the PE has slack (group 0 is PE-bound)
            g0_inject = {
                1: [(stage_m, memT, 2)],
                2: [(stage_m, memT, 3)],
                3: [(stage_m, memT, 4)],
                4: [(stage_m, memT, 5)],
                5: [(stage_m, memT, 6)],
                6: [(stage_m, memT, 7)],
                7: [(stage_x, XT, 1)],
            }

            # mm2 is emitted one triad behind mm1/ACT (carrying across
            # group boundaries) so the PE's static order runs mm1(t+1)
            # under ACT(t) instead of blocking on mm2(t)'s wait for ACT(t)
            mm2_q = []

            def emit_mm2():
                pt_prev, t_prev, nch_prev, outT_prev = mm2_q.pop(0)
                for c in range(nch_prev):
                    mc = TRI * t_prev + c
                    nc.tensor.matmul(
                        outT_prev,
                        mem_nat[:, mc, :],
                        pt_prev[:, c * NB : (c + 1) * NB],
                        start=(mc == 0),
                        stop=(mc == MCHUNKS - 1),
                        skip_group_check=True,
                    )

            # deferred sum-chain state: (g, outT, s_f32) from group end
            psum_chain = [None]

            for g in range(NG):
                xtg = XT[:, g * NB : (g + 1) * NB]
                outT = pso.tile([P, NB], F32, tag="outT")
                # two alternating bf16 triad-accumulators of exp partials
                acc = [None, None]
                duo_pt = None
                for t in range(NTRI + 1):
                    if g == 0:
                        for src, dst, s in g0_inject.get(t, []):
                            build_slice(src, dst, s)
                    elif g < NG - 1 and t == 5:
                        build_slice(stage_x, XT, g + 1)
                    if t == 2 and psum_chain[0] is not None:
                        # flush the previous group's remaining mm2 before
                        # its sum chain reads the accumulator
                        while mm2_q and mm2_q[0][3] is not outT:
                            emit_mm2()
                        emit_sum_chain(*psum_chain[0])
                        psum_chain[0] = None
                    if t == 8:
                        emit_out_chain()
                    nch = TRI if t < NTRI else 2  # trailing duo
                    ltq = psq.tile([P, nch * NB], F32, tag="lt")
                    for c in range(nch):
                        mc = TRI * t + c
                        nc.tensor.matmul(
                            ltq[:, c * NB : (c + 1) * NB],
                            memT[:, mc * P : (mc + 1) * P],
                            xtg,
                            start=True,
                            stop=True,
                        )
                    pt = ptp.tile(
                        [P, nch * NB], BF16, tag="pt" if nch == TRI else "ptd"
                    )
                    nc.scalar.activation(
                        pt, ltq, mybir.ActivationFunctionType.Exp, bias=expbias
                    )
                    mm2_q.append((pt, t, nch, outT))
                    # lag-3, and no pop in the duo iteration: keeps the
                    # boundary PE window clear so mm1(t0)/mm1(t1) of the
                    # next group run under the duo/t0 ACT ops
                    if len(mm2_q) > 3 and t != NTRI:
                        emit_mm2()
                    if nch == TRI:
                        # bf16 accumulation (2x DVE): two parities for
                        # precision, except the last group which uses a
                        # single chain so the final merge is off the tail's
                        # critical path (the adds still keep up with ACT)
                        par = t % 2 if g < NG - 1 else 0
                        a = acc[par]
                        if a is None:
                            acc[par] = pt
                        else:
                            nt = trp.tile([P, TRI * NB], BF16, tag="tr")
                            nc.vector.tensor_add(nt, a, pt)
                            acc[par] = nt
                    else:
                        duo_pt = pt
                # fold: 30 triad-chunks (3 cols) + duo (2 cols) -> [P,NB] f32
                if acc[1] is not None:
                    mrg = foldp.tile([P, TRI * NB], BF16, tag="mrg")
                    nc.vector.tensor_add(mrg, acc[0], acc[1])
                else:
                    mrg = acc[0]
                f1 = foldp.tile([P, NB], BF16, tag="f1")
                nc.vector.tensor_add(f1, mrg[:, :NB], mrg[:, NB : 2 * NB])
                f2 = foldp.tile([P, NB], BF16, tag="f2")
                nc.vector.tensor_add(f2, f1, mrg[:, 2 * NB :])
                if g < NG - 1:
                    f3 = foldp.tile([P, NB], BF16, tag="f3")
                    nc.vector.tensor_add(f3, f2, duo_pt[:, :NB])
                    s_f32 = foldp.tile([P, NB], F32, tag="fold")
                    nc.vector.tensor_add(s_f32, f3, duo_pt[:, NB:])
                    psum_chain[0] = (g, outT, s_f32)
                else:
                    psum_chain[0] = (g, outT, (f2, duo_pt))
            while mm2_q:
                emit_mm2()
            # last group: run the finalize in two column halves so the
            # gpsimd/recip/mul/transpose/copy/DMA stages pipeline instead
            # of forming one long serial tail (everything below the f2
            # fold only depends on the final duo exp)
            g_last, outT_last, (f2_last, duo_last) = psum_chain[0]
            NH = NB // 2
            for h in range(2):
                hs = slice(h * NH, (h + 1) * NH)
                f3h = foldp.tile([P, NH], BF16, tag="f3")
                nc.vector.tensor_add(f3h, f2_last[:, hs], duo_last[:, hs])
                sh = foldp.tile([P, NH], F32, tag="fold")
                nc.vector.tensor_add(sh, f3h, duo_last[:, NB + h * NH : NB + (h + 1) * NH])
                sr = sbp.tile([P, NH], F32, tag="sr")
                nc.gpsimd.partition_all_reduce(
                    sr, sh, channels=P, reduce_op=bass_isa.ReduceOp.add
                )
                rbc = sbp.tile([P, NH], F32, tag="rbc")
                nc.vector.reciprocal(rbc, sr)
                outs_sb = sbp.tile([P, NH], F32, tag="outs")
                nc.vector.tensor_mul(outs_sb, outT_last[:, hs], rbc)
                onat = psb.tile([P, NH], F32, tag="tp")
                for j in range(NH // P):
                    nc.tensor.transpose(
                        onat[:, j * P : (j + 1) * P],
                        outs_sb[:, j * P : (j + 1) * P],
                        ident,
                    )
                out_sb = sbp.tile([P, NH], F32, tag="osb")
                nc.vector.tensor_copy(out=out_sb, in_=onat)
                nc.sync.dma_start(
                    out=out[
                        g_last * NB + h * NH : g_last * NB + (h + 1) * NH, :
                    ].rearrange("(j p) d -> p j d", p=P),
                    in_=out_sb.rearrange("p (j d) -> p j d", d=D),
                )

    nc.compile()
    return nc


_NC_CACHE = None


def _get_nc():
    global _NC_CACHE
    if _NC_CACHE is None:
        _NC_CACHE = build_nc()
    return _NC_CACHE


def _in_maps(local_stats, memory):
    local_stats = np.ascontiguousarray(local_stats, dtype=np.float32)
    memory = np.ascontiguousarray(memory, dtype=np.float32)
    return [
        {
            "x": np.ascontiguousarray(local_stats[i * BLOC : (i + 1) * BLOC]),
            "mem": memory,
        }
        for i in range(NCORES)
    ]


def run_spmd(local_stats, memory, **kwargs):
    """Run on all 8 cores; returns BassKernelResults (for test harness use)."""
    from concourse.bass_utils import run_bass_kernel_spmd

    nc = _get_nc()
    return run_bass_kernel_spmd(
        nc, _in_maps(local_stats, memory), core_ids=list(range(NCORES)), **kwargs
    )


def kernel(local_stats, memory):
    res = run_spmd(local_stats, memory)
    return np.concatenate([r["out"] for r in res.results], axis=0)
